# revision 4
# baseline (speedup 1.0000x reference)
"""Distributed causal multi-head attention for 8 TRN2 NeuronCores.

Problem: x[4,2048,1024] -> qkv proj -> 16-head causal attention -> out proj.
Sharding (Megatron TP over heads x DP over batch): core i handles batch
b=i//2 and head group g=i%2 (8 heads of 64 dims each). qkv weights are
column-sliced, proj weights row-sliced; the two cores of a batch pair
combine their partial projection outputs with an on-device pairwise
ReduceScatter, so core 2b returns tokens 0..1023 and core 2b+1 tokens
1024..2047 of batch b.

Per-core compute (all matmuls bf16, fp32 PSUM accumulation):
  QK^T = w_qk^T @ x^T  (features on partitions -> Q^T,K^T layout [dh, t])
  V    = x @ w_v       (tokens on partitions, with a ones-column appended)
  S^T  = K^T.T @ Q^T   per head/kv-chunk -> exp -> causal mask-mult
  O^T_aug = V_aug.T @ P^T  (row 64 = softmax denominators, ones-col trick)
  O^T  = O^T_aug * (1/denom) (DMA partition-broadcast of the recip row)
  part = O^T.T @ pw (+ proj bias on even cores) -> ReduceScatter(pair)
"""
import os
import sys
import types

sys.path.insert(0, "/opt/trn_rl_repo")

import numpy as np
import ml_dtypes

# ---------------------------------------------------------------------------
# antenv.axon_hooks shim: enables NTFF profiling (trace=True) under axon.
# Harmless when tracing is off.
# ---------------------------------------------------------------------------
def _install_ntff_shim():
    try:
        import antenv
    except ImportError:
        return
    if "antenv.axon_hooks" in sys.modules:
        return
    mod = types.ModuleType("antenv.axon_hooks")
    mod._hook = None
    def set_axon_ntff_profile_hook(h):
        mod._hook = h
    def get_axon_ntff_profile_hook():
        return mod._hook
    mod.set_axon_ntff_profile_hook = set_axon_ntff_profile_hook
    mod.get_axon_ntff_profile_hook = get_axon_ntff_profile_hook
    sys.modules["antenv.axon_hooks"] = mod
    antenv.axon_hooks = mod
    try:
        from trn_agent_boot.trn_boot import _ntff_profile_via_ctypes
        hook = _ntff_profile_via_ctypes("/opt/axon/libaxon_pjrt.so")
        if hook is not None:
            set_axon_ntff_profile_hook(hook)
    except Exception:
        pass


_install_ntff_shim()

import concourse.bass as bass
import concourse.mybir as mybir
import concourse.tile as tile
from concourse import bacc
from concourse import bass_utils

bass_utils.upload_artifacts = lambda tmpdir: "local://skipped"
from concourse.bass_utils import run_bass_kernel_spmd

BF16 = mybir.dt.bfloat16
F32 = mybir.dt.float32
NP_BF16 = ml_dtypes.bfloat16

B, T, C = 4, 2048, 1024
N_HEADS_LOCAL = 8          # heads per core
DH = 64
HD = N_HEADS_LOCAL * DH    # 512 local head dims
N_CORES = 8
GROUPS = [[0, 1], [2, 3], [4, 5], [6, 7]]
QT = 512                   # q tile (free dim)
KC = 128                   # kv chunk (partitions)
N_QT = T // QT             # 4
N_KC = T // KC             # 16
VW = DH + 1                # v tile width per head incl. ones column


def build_graph():
    nc = bacc.Bacc("TRN2", target_bir_lowering=False, debug=False,
                   enable_asserts=True, num_devices=N_CORES)

    xT_ext = nc.dram_tensor("xT", [C, T], BF16, kind="ExternalInput").ap()
    wqk_ext = nc.dram_tensor("w_qk", [C, 2 * HD], BF16, kind="ExternalInput").ap()
    wv_ext = nc.dram_tensor("w_v", [C, HD], BF16, kind="ExternalInput").ap()
    bqk_ext = nc.dram_tensor("b_qk", [128, 8], F32, kind="ExternalInput").ap()
    vb_ext = nc.dram_tensor("vb", [128, HD], F32, kind="ExternalInput").ap()
    pw_ext = nc.dram_tensor("pw", [HD, C], BF16, kind="ExternalInput").ap()
    pb_ext = nc.dram_tensor("pb", [128, C], F32, kind="ExternalInput").ap()
    mask_ext = nc.dram_tensor("masks", [4, KC, QT], BF16, kind="ExternalInput").ap()
    out_ext = nc.dram_tensor("out", [T // 2, C], F32, kind="ExternalOutput").ap()

    partial_dram = nc.dram_tensor("partial", [T, C], F32).ap()
    rs_dram = nc.dram_tensor("rs_out", [T // 2, C], F32).ap()

    with tile.TileContext(nc) as tc:
        with (
            tc.tile_pool(name="persist", bufs=1) as persist,
            tc.tile_pool(name="pt_pool", bufs=18) as pt_pool,
            tc.tile_pool(name="small", bufs=4) as small,
            tc.tile_pool(name="outp", bufs=3) as outp,
            tc.tile_pool(name="ps_s", bufs=3, space="PSUM") as ps_s,
            tc.tile_pool(name="ps_acc", bufs=2, space="PSUM") as ps_acc,
            tc.tile_pool(name="ps_o", bufs=2, space="PSUM") as ps_o,
        ):
            # ---- resident weights / constants -----------------------------
            wqk_sb = persist.tile([128, 8, 2 * HD], BF16, tag="wqk")
            nc.sync.dma_start(wqk_sb[:], wqk_ext.rearrange("(o p) n -> p o n", p=128))
            wv_sb = persist.tile([128, 8, HD], BF16, tag="wv")
            nc.sync.dma_start(wv_sb[:], wv_ext.rearrange("(o p) n -> p o n", p=128))
            pw_sb = persist.tile([128, 4, C], BF16, tag="pw")
            nc.sync.dma_start(pw_sb[:], pw_ext.rearrange("(o p) n -> p o n", p=128))
            bqk_sb = persist.tile([128, 8], F32, tag="bqk")
            nc.sync.dma_start(bqk_sb[:], bqk_ext[:])
            vb_sb = persist.tile([128, HD], F32, tag="vb")
            nc.sync.dma_start(vb_sb[:], vb_ext[:])
            pb_sb = persist.tile([128, C], F32, tag="pb")
            nc.sync.dma_start(pb_sb[:], pb_ext[:])
            mask_sb = persist.tile([128, 4, QT], BF16, tag="mask")
            nc.sync.dma_start(mask_sb[:], mask_ext.rearrange("v p q -> p v q"))
            xT_sb = persist.tile([128, 8, T], BF16, tag="xT")
            nc.sync.dma_start(xT_sb[:], xT_ext.rearrange("(o p) t -> p o t", p=128))

            # ---- QK^T projection: psum[qk 128, t 512] ---------------------
            # feature chunk o (0..3 = Q of heads 2o,2o+1; 4..7 = K)
            qkt = [persist.tile([128, T], BF16, tag=f"qkt{o}", name=f"qkt{o}") for o in range(8)]
            for o in range(8):
                for tt in range(N_QT):
                    ps = ps_acc.tile([128, QT], F32, tag="acc128")
                    for cc in range(8):
                        nc.tensor.matmul(
                            ps[:],
                            wqk_sb[:, cc, o * 128:(o + 1) * 128],
                            xT_sb[:, cc, tt * QT:(tt + 1) * QT],
                            start=(cc == 0), stop=(cc == 7),
                        )
                    nc.vector.tensor_add(
                        qkt[o][:, tt * QT:(tt + 1) * QT], ps[:],
                        bqk_sb[:, o:o + 1].to_broadcast((128, QT)),
                    )

            # ---- V projection: psum[t 128, hd 512], + ones column ---------
            vt = [persist.tile([128, N_HEADS_LOCAL, VW], BF16, tag=f"v{kc}", name=f"v{kc}")
                  for kc in range(N_KC)]
            for kc in range(N_KC):
                ps = ps_acc.tile([128, HD], F32, tag="acc128")
                for cc in range(8):
                    nc.tensor.matmul(
                        ps[:],
                        xT_sb[:, cc, kc * KC:(kc + 1) * KC],
                        wv_sb[:, cc, :],
                        start=(cc == 0), stop=(cc == 7),
                    )
                nc.vector.tensor_add(
                    vt[kc][:, :, 0:DH],
                    ps[:].rearrange("p (h d) -> p h d", h=N_HEADS_LOCAL),
                    vb_sb[:].rearrange("p (h d) -> p h d", h=N_HEADS_LOCAL),
                )
                nc.any.memset(vt[kc][:, :, DH:VW], 1.0)

            # ---- attention + output accumulation --------------------------
            # O^T stored per feature chunk: ot[hc][ (h%2)*64 : +64, t ]
            ot = [persist.tile([128, T], BF16, tag=f"ot{hc}", name=f"ot{hc}") for hc in range(4)]
            for h in range(N_HEADS_LOCAL):
                hc, hp = h // 2, (h % 2) * 64
                for qt in range(N_QT):
                    nkc = 4 * qt + 4
                    pts = []
                    for kc in range(nkc):
                        ps = ps_s.tile([128, QT], F32, tag="s")
                        nc.tensor.matmul(
                            ps[:],
                            qkt[4 + hc][hp:hp + 64, kc * KC:(kc + 1) * KC],
                            qkt[hc][hp:hp + 64, qt * QT:(qt + 1) * QT],
                            start=True, stop=True,
                        )
                        pt = pt_pool.tile([128, QT], BF16, tag="pt")
                        nc.scalar.activation(pt[:], ps[:],
                                             mybir.ActivationFunctionType.Exp)
                        v = kc - 4 * qt
                        if v >= 0:
                            nc.vector.tensor_mul(pt[:], pt[:], mask_sb[:, v, :])
                        pts.append(pt)
                    po = ps_o.tile([VW, QT], F32, tag="o")
                    for kc in range(nkc):
                        nc.tensor.matmul(
                            po[:], vt[kc][:, h, :], pts[kc][:],
                            start=(kc == 0), stop=(kc == nkc - 1),
                        )
                    den = small.tile([1, QT], F32, tag="den")
                    nc.vector.tensor_copy(den[:], po[DH:VW, :])
                    recip = small.tile([1, QT], F32, tag="recip")
                    nc.vector.reciprocal_approx_fast(recip[:], den[:])
                    bcast = small.tile([64, QT], F32, tag="bcast")
                    nc.gpsimd.partition_broadcast(bcast[:], recip[:])
                    nc.vector.tensor_mul(
                        ot[hc][hp:hp + 64, qt * QT:(qt + 1) * QT],
                        po[0:DH, :], bcast[:],
                    )

            # ---- output projection + partial store ------------------------
            for tt in range(N_KC):
                for ct in range(2):
                    ps = ps_acc.tile([128, QT], F32, tag="acc128")
                    for hc in range(4):
                        nc.tensor.matmul(
                            ps[:],
                            ot[hc][:, tt * KC:(tt + 1) * KC],
                            pw_sb[:, hc, ct * QT:(ct + 1) * QT],
                            start=(hc == 0), stop=(hc == 3),
                        )
                    po = outp.tile([128, QT], F32, tag="po")
                    nc.vector.tensor_add(po[:], ps[:], pb_sb[:, ct * QT:(ct + 1) * QT])
                    nc.sync.dma_start(
                        partial_dram[tt * KC:(tt + 1) * KC, ct * QT:(ct + 1) * QT],
                        po[:],
                    )

            # ---- pairwise ReduceScatter + output --------------------------
            nc.gpsimd.collective_compute(
                "ReduceScatter",
                mybir.AluOpType.add,
                replica_groups=GROUPS,
                ins=[partial_dram[:]],
                outs=[rs_dram[:]],
            )
            nc.sync.dma_start(out_ext[:], rs_dram[:])

    nc.compile()
    return nc


_NC = None


def _get_nc():
    global _NC
    if _NC is None:
        _NC = build_graph()
    return _NC


def make_masks():
    # masks[v][k_l, q_l] = 1 where (128*v + k_l) <= q_l else 0
    v = np.arange(4)[:, None, None]
    k = np.arange(KC)[None, :, None]
    q = np.arange(QT)[None, None, :]
    return ((128 * v + k) <= q).astype(np.float32)


def shard_inputs(x, qkv_w, qkv_b, proj_w, proj_b):
    x = np.asarray(x, np.float32)
    qkv_w = np.asarray(qkv_w, np.float32)
    qkv_b = np.asarray(qkv_b, np.float32)
    proj_w = np.asarray(proj_w, np.float32)
    proj_b = np.asarray(proj_b, np.float32)
    scale = DH ** (-0.5)
    masks = make_masks().astype(NP_BF16)
    in_maps = []
    for i in range(N_CORES):
        b, g = i // 2, i % 2
        sl = slice(g * HD, (g + 1) * HD)
        w_qk = np.concatenate(
            [qkv_w[:, 0:C][:, sl] * scale, qkv_w[:, C:2 * C][:, sl]], axis=1)
        b_cat = np.concatenate(
            [qkv_b[0:C][sl] * scale, qkv_b[C:2 * C][sl]])
        in_maps.append({
            "xT": np.ascontiguousarray(x[b].T).astype(NP_BF16),
            "w_qk": w_qk.astype(NP_BF16),
            "w_v": qkv_w[:, 2 * C:3 * C][:, sl].astype(NP_BF16),
            "b_qk": np.ascontiguousarray(b_cat.reshape(8, 128).T),
            "vb": np.tile(qkv_b[2 * C:3 * C][sl][None, :], (128, 1)),
            "pw": proj_w[sl, :].astype(NP_BF16),
            "pb": (np.tile(proj_b[None, :], (128, 1)) if g == 0
                   else np.zeros((128, C), np.float32)),
            "masks": masks,
        })
    return in_maps


LAST_RESULT = None


def kernel(x, qkv_w, qkv_b, proj_w, proj_b):
    global LAST_RESULT
    nc = _get_nc()
    in_maps = shard_inputs(x, qkv_w, qkv_b, proj_w, proj_b)
    trace = bool(int(os.environ.get("BASS_KERNEL_TRACE", "0")))
    kwargs = {}
    if trace:
        kwargs = dict(trace=True, trace_cores=list(range(N_CORES)))
    res = run_bass_kernel_spmd(nc, in_maps, core_ids=list(range(N_CORES)), **kwargs)
    LAST_RESULT = res
    out = np.empty((B, T, C), np.float32)
    for i in range(N_CORES):
        b, g = i // 2, i % 2
        out[b, g * (T // 2):(g + 1) * (T // 2), :] = res.results[i]["out"]
    return out


# revision 10
# speedup vs baseline: 1.3040x; 1.3040x over previous
"""Distributed causal multi-head attention for 8 TRN2 NeuronCores.

Problem: x[4,2048,1024] -> qkv proj -> 16-head causal attention -> out proj.
Sharding (Megatron TP over heads x DP over batch): core i handles batch
b=i//2 and head group g=i%2 (8 heads of 64 dims each). qkv weights are
column-sliced, proj weights row-sliced; the two cores of a batch pair
combine their partial projection outputs with an on-device pairwise
ReduceScatter, so core 2b returns tokens 0..1023 and core 2b+1 tokens
1024..2047 of batch b.

Per-core compute (all matmuls bf16, fp32 PSUM accumulation):
  QK^T = w_qk^T @ x^T  (features on partitions -> Q^T,K^T layout [dh, t])
  V    = x @ w_v       (tokens on partitions, with a ones-column appended)
  S^T  = K^T.T @ Q^T   per head/kv-chunk -> exp -> causal mask-mult
  O^T_aug = V_aug.T @ P^T  (row 64 = softmax denominators, ones-col trick)
  O^T  = O^T_aug * (1/denom) (DMA partition-broadcast of the recip row)
  part = O^T.T @ pw (+ proj bias on even cores) -> ReduceScatter(pair)
"""
import os
import sys
import types

sys.path.insert(0, "/opt/trn_rl_repo")

import numpy as np
import ml_dtypes

# ---------------------------------------------------------------------------
# antenv.axon_hooks shim: enables NTFF profiling (trace=True) under axon.
# Harmless when tracing is off.
# ---------------------------------------------------------------------------
def _install_ntff_shim():
    try:
        import antenv
    except ImportError:
        return
    if "antenv.axon_hooks" in sys.modules:
        return
    mod = types.ModuleType("antenv.axon_hooks")
    mod._hook = None
    def set_axon_ntff_profile_hook(h):
        mod._hook = h
    def get_axon_ntff_profile_hook():
        return mod._hook
    mod.set_axon_ntff_profile_hook = set_axon_ntff_profile_hook
    mod.get_axon_ntff_profile_hook = get_axon_ntff_profile_hook
    sys.modules["antenv.axon_hooks"] = mod
    antenv.axon_hooks = mod
    try:
        from trn_agent_boot.trn_boot import _ntff_profile_via_ctypes
        hook = _ntff_profile_via_ctypes("/opt/axon/libaxon_pjrt.so")
        if hook is not None:
            set_axon_ntff_profile_hook(hook)
    except Exception:
        pass


_install_ntff_shim()

import concourse.bass as bass
import concourse.mybir as mybir
import concourse.tile as tile
from concourse import bacc
from concourse import bass_utils

bass_utils.upload_artifacts = lambda tmpdir: "local://skipped"
from concourse.bass_utils import run_bass_kernel_spmd

BF16 = mybir.dt.bfloat16
F32 = mybir.dt.float32
NP_BF16 = ml_dtypes.bfloat16

B, T, C = 4, 2048, 1024
N_HEADS_LOCAL = 8          # heads per core
DH = 64
HD = N_HEADS_LOCAL * DH    # 512 local head dims
N_CORES = 8
GROUPS = [[0, 1], [2, 3], [4, 5], [6, 7]]
QT = 512                   # q tile (free dim)
KC = 128                   # kv chunk (partitions)
N_QT = T // QT             # 4
N_KC = T // KC             # 16
VW = DH + 1                # v tile width per head incl. ones column


def build_graph():
    nc = bacc.Bacc("TRN2", target_bir_lowering=False, debug=False,
                   enable_asserts=True, num_devices=N_CORES)

    xT_ext = nc.dram_tensor("xT", [C, T], BF16, kind="ExternalInput").ap()
    wqk_ext = nc.dram_tensor("w_qk", [C, 2 * HD], BF16, kind="ExternalInput").ap()
    wv_ext = nc.dram_tensor("w_v", [C, HD], BF16, kind="ExternalInput").ap()
    bqk_ext = nc.dram_tensor("b_qk", [128, 8], F32, kind="ExternalInput").ap()
    vb_ext = nc.dram_tensor("vb", [128, HD], F32, kind="ExternalInput").ap()
    pw_ext = nc.dram_tensor("pw", [HD, C], BF16, kind="ExternalInput").ap()
    pb_ext = nc.dram_tensor("pb", [128, C], F32, kind="ExternalInput").ap()
    mask_ext = nc.dram_tensor("masks", [4, KC, QT], BF16, kind="ExternalInput").ap()
    out_ext = nc.dram_tensor("out", [T // 2, C], F32, kind="ExternalOutput").ap()

    partial_dram = nc.dram_tensor("partial", [T, C], F32).ap()
    rs_dram = nc.dram_tensor("rs_out", [T // 2, C], F32).ap()

    with tile.TileContext(nc) as tc:
        with (
            tc.tile_pool(name="persist", bufs=1) as persist,
            tc.tile_pool(name="ps_s", bufs=4, space="PSUM") as ps_s,
            tc.tile_pool(name="ps_acc", bufs=2, space="PSUM") as ps_acc,
            tc.tile_pool(name="ps_o", bufs=2, space="PSUM") as ps_o,
        ):
            # ---- resident weights / constants -----------------------------
            # chunked DMAs so the 16 SDMA queues run in parallel
            wqk_sb = persist.tile([128, 8, 2 * HD], BF16, tag="wqk")
            wqk_r = wqk_ext.rearrange("(o p) n -> p o n", p=128)
            for cc in range(8):
                nc.sync.dma_start(wqk_sb[:, cc:cc + 1, :], wqk_r[:, cc:cc + 1, :])
            wv_sb = persist.tile([128, 8, HD], BF16, tag="wv")
            wv_r = wv_ext.rearrange("(o p) n -> p o n", p=128)
            for cc in range(0, 8, 2):
                nc.sync.dma_start(wv_sb[:, cc:cc + 2, :], wv_r[:, cc:cc + 2, :])
            pw_sb = persist.tile([128, 4, C], BF16, tag="pw")
            nc.sync.dma_start(pw_sb[:], pw_ext.rearrange("(o p) n -> p o n", p=128))
            bqk_sb = persist.tile([128, 8], F32, tag="bqk")
            nc.sync.dma_start(bqk_sb[:], bqk_ext[:])
            vb_sb = persist.tile([128, HD], F32, tag="vb")
            nc.sync.dma_start(vb_sb[:], vb_ext[:])
            pb_sb = persist.tile([128, C], F32, tag="pb")
            nc.sync.dma_start(pb_sb[:], pb_ext[:])
            mask_sb = persist.tile([128, 4, QT], BF16, tag="mask")
            nc.sync.dma_start(mask_sb[:], mask_ext.rearrange("v p q -> p v q"))
            xtp_cm = tc.tile_pool(name="xtp", bufs=1)
            xtp = xtp_cm.__enter__()
            xT_sb = xtp.tile([128, 8, T], BF16, tag="xT")
            xT_r = xT_ext.rearrange("(o p) t -> p o t", p=128)
            for cc in range(8):
                nc.sync.dma_start(xT_sb[:, cc:cc + 1, :], xT_r[:, cc:cc + 1, :])

            # ---- QK^T projection: psum[qk 128, t 512] ---------------------
            # feature chunk o (0..3 = Q of heads 2o,2o+1; 4..7 = K)
            qkt = [persist.tile([128, T], BF16, tag=f"qkt{o}", name=f"qkt{o}") for o in range(8)]
            for o in range(8):
                for tt in range(N_QT):
                    ps = ps_acc.tile([128, QT], F32, tag="acc128")
                    for cc in range(8):
                        nc.tensor.matmul(
                            ps[:],
                            wqk_sb[:, cc, o * 128:(o + 1) * 128],
                            xT_sb[:, cc, tt * QT:(tt + 1) * QT],
                            start=(cc == 0), stop=(cc == 7),
                        )
                    nc.vector.tensor_add(
                        qkt[o][:, tt * QT:(tt + 1) * QT], ps[:],
                        bqk_sb[:, o:o + 1].to_broadcast((128, QT)),
                    )

            # ---- V projection: psum[t 128, hd 512], + ones column ---------
            vt = [persist.tile([128, N_HEADS_LOCAL, VW], BF16, tag=f"v{kc}", name=f"v{kc}")
                  for kc in range(N_KC)]
            for kc in range(N_KC):
                ps = ps_acc.tile([128, HD], F32, tag="acc128")
                for cc in range(8):
                    nc.tensor.matmul(
                        ps[:],
                        xT_sb[:, cc, kc * KC:(kc + 1) * KC],
                        wv_sb[:, cc, :],
                        start=(cc == 0), stop=(cc == 7),
                    )
                nc.vector.tensor_add(
                    vt[kc][:, :, 0:DH],
                    ps[:].rearrange("p (h d) -> p h d", h=N_HEADS_LOCAL),
                    vb_sb[:].rearrange("p (h d) -> p h d", h=N_HEADS_LOCAL),
                )
                nc.any.memset(vt[kc][:, :, DH:VW], 1.0)

            xtp_cm.__exit__(None, None, None)
            pt_cm = tc.tile_pool(name="pt_pool", bufs=36)
            pt_pool = pt_cm.__enter__()
            small_cm = tc.tile_pool(name="small", bufs=6)
            small = small_cm.__enter__()
            outp_cm = tc.tile_pool(name="outp", bufs=3)
            outp = outp_cm.__enter__()

            # ---- attention + chunked projection / ReduceScatter -----------
            # qt-outer so each 512-token block's projection + pairwise RS
            # overlaps the next block's attention. Score matmuls for the two
            # heads of a feature chunk are row-packed (tile_position) so the
            # K=64 pairs run concurrently in the PE array.
            # O^T per feature chunk: ot[hc][ (h%2)*64 : +64, t ]
            ot = [persist.tile([128, T], BF16, tag=f"ot{hc}", name=f"ot{hc}") for hc in range(4)]
            for qt in range(N_QT):
                nkc = 4 * qt + 4
                qsl = slice(qt * QT, (qt + 1) * QT)
                for hc in range(4):
                    pts = {0: [], 1: []}
                    for kc in range(nkc):
                        ksl = slice(kc * KC, (kc + 1) * KC)
                        v = kc - 4 * qt
                        for sub in range(2):
                            hp = sub * 64
                            ps = ps_s.tile([128, QT], F32, tag="s")
                            nc.tensor.matmul(
                                ps[:],
                                qkt[4 + hc][hp:hp + 64, ksl],
                                qkt[hc][hp:hp + 64, qsl],
                                start=True, stop=True,
                                tile_position=(hp, 0),
                            )
                            pt = pt_pool.tile([128, QT], BF16, tag="pt")
                            nc.scalar.activation(pt[:], ps[:],
                                                 mybir.ActivationFunctionType.Exp)
                            if v >= 0:
                                nc.vector.tensor_mul(pt[:], pt[:], mask_sb[:, v, :])
                            pts[sub].append(pt)
                    for sub in range(2):
                        h = 2 * hc + sub
                        hp = sub * 64
                        po = ps_o.tile([VW, QT], F32, tag="o")
                        for kc in range(nkc):
                            nc.tensor.matmul(
                                po[:], vt[kc][:, h, :], pts[sub][kc][:],
                                start=(kc == 0), stop=(kc == nkc - 1),
                            )
                        den = small.tile([1, QT], F32, tag="den")
                        nc.vector.tensor_copy(den[:], po[DH:VW, :])
                        recip = small.tile([1, QT], F32, tag="recip")
                        nc.vector.reciprocal_approx_fast(recip[:], den[:])
                        bcast = small.tile([64, QT], F32, tag="bcast")
                        nc.gpsimd.partition_broadcast(bcast[:], recip[:])
                        nc.vector.tensor_mul(
                            ot[hc][hp:hp + 64, qsl],
                            po[0:DH, :], bcast[:],
                        )

                # ---- projection + partial store for this 512-token block --
                for tsub in range(QT // KC):
                    tt = qt * (QT // KC) + tsub
                    for ct in range(2):
                        ps = ps_acc.tile([128, QT], F32, tag="acc128")
                        for hc in range(4):
                            nc.tensor.matmul(
                                ps[:],
                                ot[hc][:, tt * KC:(tt + 1) * KC],
                                pw_sb[:, hc, ct * QT:(ct + 1) * QT],
                                start=(hc == 0), stop=(hc == 3),
                            )
                        po = outp.tile([128, QT], F32, tag="po")
                        nc.vector.tensor_add(po[:], ps[:],
                                             pb_sb[:, ct * QT:(ct + 1) * QT])
                        nc.sync.dma_start(
                            partial_dram[tt * KC:(tt + 1) * KC,
                                         ct * QT:(ct + 1) * QT],
                            po[:],
                        )
                # ---- pairwise ReduceScatter for this block (overlapped) ---
                nc.gpsimd.collective_compute(
                    "ReduceScatter",
                    mybir.AluOpType.add,
                    replica_groups=GROUPS,
                    ins=[partial_dram[qsl, :]],
                    outs=[rs_dram[qt * (QT // 2):(qt + 1) * (QT // 2), :]],
                )
                nc.sync.dma_start(
                    out_ext[qt * (QT // 2):(qt + 1) * (QT // 2), :],
                    rs_dram[qt * (QT // 2):(qt + 1) * (QT // 2), :],
                )
            outp_cm.__exit__(None, None, None)
            small_cm.__exit__(None, None, None)
            pt_cm.__exit__(None, None, None)

    nc.compile()
    return nc


_NC = None


def _get_nc():
    global _NC
    if _NC is None:
        _NC = build_graph()
    return _NC


def make_masks():
    # masks[v][k_l, q_l] = 1 where (128*v + k_l) <= q_l else 0
    v = np.arange(4)[:, None, None]
    k = np.arange(KC)[None, :, None]
    q = np.arange(QT)[None, None, :]
    return ((128 * v + k) <= q).astype(np.float32)


def shard_inputs(x, qkv_w, qkv_b, proj_w, proj_b):
    x = np.asarray(x, np.float32)
    qkv_w = np.asarray(qkv_w, np.float32)
    qkv_b = np.asarray(qkv_b, np.float32)
    proj_w = np.asarray(proj_w, np.float32)
    proj_b = np.asarray(proj_b, np.float32)
    scale = DH ** (-0.5)
    masks = make_masks().astype(NP_BF16)
    in_maps = []
    for i in range(N_CORES):
        b, g = i // 2, i % 2
        sl = slice(g * HD, (g + 1) * HD)
        w_qk = np.concatenate(
            [qkv_w[:, 0:C][:, sl] * scale, qkv_w[:, C:2 * C][:, sl]], axis=1)
        b_cat = np.concatenate(
            [qkv_b[0:C][sl] * scale, qkv_b[C:2 * C][sl]])
        in_maps.append({
            "xT": np.ascontiguousarray(x[b].T).astype(NP_BF16),
            "w_qk": w_qk.astype(NP_BF16),
            "w_v": qkv_w[:, 2 * C:3 * C][:, sl].astype(NP_BF16),
            "b_qk": np.ascontiguousarray(b_cat.reshape(8, 128).T),
            "vb": np.tile(qkv_b[2 * C:3 * C][sl][None, :], (128, 1)),
            "pw": proj_w[sl, :].astype(NP_BF16),
            "pb": (np.tile(proj_b[None, :], (128, 1)) if g == 0
                   else np.zeros((128, C), np.float32)),
            "masks": masks,
        })
    return in_maps


LAST_RESULT = None


def kernel(x, qkv_w, qkv_b, proj_w, proj_b):
    global LAST_RESULT
    nc = _get_nc()
    in_maps = shard_inputs(x, qkv_w, qkv_b, proj_w, proj_b)
    trace = bool(int(os.environ.get("BASS_KERNEL_TRACE", "0")))
    kwargs = {}
    if trace:
        kwargs = dict(trace=True, trace_cores=list(range(N_CORES)))
    res = run_bass_kernel_spmd(nc, in_maps, core_ids=list(range(N_CORES)), **kwargs)
    LAST_RESULT = res
    out = np.empty((B, T, C), np.float32)
    half = QT // 2
    for i in range(N_CORES):
        b, g = i // 2, i % 2
        o = res.results[i]["out"]
        for qt in range(N_QT):
            out[b, qt * QT + g * half: qt * QT + (g + 1) * half, :] = \
                o[qt * half:(qt + 1) * half]
    return out
